# revision 11
# baseline (speedup 1.0000x reference)
"""Trainium2 Bass kernel for nn_AdditionFFN (4-step byte-addition FFN).

Reference semantics: 4 sequential steps; step i forms x = [a_i, b_i, carry]
(len 514), takes softmax(10*(x @ W1 - 2.5)) over 131072 one-hot table
entries, then result_i = weights @ W2_sum and carry' = weights @ W2_carry.

The tables are the deterministic one-hot structures from the reference's
_build_tables() (entry idx = a*512 + b*2 + c scores
a_emb[i,a] + b_emb[i,b] + carry[c]); kernel() verifies this structure
exactly and falls back to a direct on-device jax evaluation if it ever
fails to hold.  Under that structure the softmax factorizes:

    exp-scores = (ea (x) eb) (x) [e^{c0}, e^{c1}],  ea/eb = exp(10*emb - 12.5)

so weights @ W2_sum collapses to a 256-point circular convolution
u = ea (*) eb plus a roll by one for the odd-parity (carry-in) half, and
the carry chain reduces to a scalar recurrence over t_i = tanh(s_i/2)
(sigma_i = (1+t_i)/2 is the carry-in-1 mass):

    Z_i   = sum(ea_i) * sum(eb_i)            (= sum(u_i))
    p1_i  = sum_{a+b>=256} ea[a] eb[b]       (carry-out mass, c=0 part)
    A''_i = (10*p1_i + 5*u_i[255])/Z_i - 5 ;  B''_i = 5*u_i[255]/Z_i
    t_{i+1} = tanh(B''_i * t_i + A''_i),      t_0 = tanh(-5)
    out_i = (1-sg_i)/Z_i * u_i + sg_i/Z_i * roll(u_i, 1),  sg = (1+t)/2

Device implementation (SPMD on 8 NeuronCores, no collectives - every core
computes the full answer redundantly and core 0's output is returned):

  - circular conv as TensorE correlation: lhsT row q = ea[255-q]
    (block-diagonal bf16 [128, 4] chunks, written DIRECTLY by the a-side
    exp through a strided output AP); one overlapping-run DMA of the
    host-cast fp16 [4, 512] copy gives Wall[p, m] = flat[p + m], so ONE
    strided ACTIVATE exps all four step windows at once.  The psum is
    [4, 257] wide: col j = u[(j-1) mod 256] - roll(u,1) is P[:, 0:256]
    and u is P[:, 1:257] with no wrap fixup.
  - ALL softmax statistics come from host-packed rows R = [a_rev | b]
    ([4, 512], exp'd once on ACT): u255 = dot(ea_rev, eb_row) (Pool STT
    accumulate), PrefB = inclusive cumsum of eb_row (DVE scan),
    p1 = Z - dot(ea_rev, PrefB), Z = sum(ea) * PrefB[255].  No second
    matmul group and no on-device one-hot tables at all.
  - carry chain: t' = tanh(B''*t + A'') folds each recurrence step into a
    single ACTIVATE (per-instruction scale/bias APs); tanh lives in the
    same ACT table set as exp, so there is exactly one table load.
  - final combine is two DVE STTs with per-partition scalars prescaled by
    1/Z on the idle Pool engine: alpha = (1-sg)/(2...)/Z, beta = sg/Z:
    o1 = alpha*u, out = beta*roll + o1 (PSUM is read once per STT, which
    the s2s2d2 datapath requires anyway).
  - six back-to-back [4,512] dummy matmuls open the PE HAM clock gate
    (~3.4us of continuous activity) so the 8 real conv matmuls run warm.
  - a dummy DMA keyed on the chain keeps the sync HWDGE queue awake so
    the output DMA skips the ~1us queue wake.
"""

import sys

sys.path.insert(0, "/opt/trn_rl_repo")

import numpy as np

import concourse.bacc as bacc
import concourse.mybir as mybir
import concourse.tile as tile
from concourse.ap import AP
from concourse.bass_utils import run_bass_kernel_spmd

N_CORES = 8
D = 256
F32 = mybir.dt.float32
BF16 = mybir.dt.bfloat16
FP16 = mybir.dt.float16
EXP = mybir.ActivationFunctionType.Exp
TANH = mybir.ActivationFunctionType.Tanh
MULT = mybir.AluOpType.mult
ADD = mybir.AluOpType.add
BYPASS = mybir.AluOpType.bypass
T0 = float(np.tanh(-5.0))


def build_nc():
    nc = bacc.Bacc(None, target_bir_lowering=False, debug=False,
                   enable_partition_id=False)

    apack = nc.declare_dram_parameter("apack", [128, 8], F32, isOutput=False)
    rows = nc.declare_dram_parameter("rows", [4, 2 * D], F32, isOutput=False)
    b2h = nc.declare_dram_parameter("b2h", [4, 2 * D], FP16, isOutput=False)
    out = nc.declare_dram_parameter("out", [4, D], FP16, isOutput=True)

    with tile.TileContext(nc) as tc:
        with (
            tc.tile_pool(name="pool", bufs=1) as pool,
            tc.tile_pool(name="psum", bufs=1, space="PSUM") as psum,
        ):
            # --- constants.  dmy first on gpsimd: it gates the PE warmup.
            dmy = pool.tile([128, 2 * D], BF16, tag="dmy")
            nc.gpsimd.memset(dmy[:], 1.0)
            bd = pool.tile([128, 32], BF16, tag="bd")
            nc.gpsimd.memset(bd[:], 0.0)
            bias128 = pool.tile([128, 1], F32, tag="bias128")
            nc.vector.memset(bias128[:], -12.5)
            bias4 = pool.tile([4, 1], F32, tag="bias4")
            nc.vector.memset(bias4[:], -12.5)
            stg1 = pool.tile([32, 96], F32, tag="stg1")
            nc.vector.memset(stg1[:], 0.0)
            stg2 = pool.tile([32, 32], F32, tag="stg2")
            nc.vector.memset(stg2[:], 0.0)
            nc.vector.memset(stg2[0:1, 0:1], T0)
            zrow = pool.tile([4, D], F32, tag="zrow")
            nc.vector.memset(zrow[:], 0.0)

            # --- input DMAs: two parallel HWDGE queues (queue first-byte is
            # ~1.7us after dispatch, so the tiny stats tensors go first and
            # the four step-windows stream behind them, one half-partition
            # transfer per queue per step: W_i[p, f] = flat[512i + p + f].
            apackT = pool.tile([128, 8], F32, tag="apackT")
            nc.sync.dma_start(apackT[:], apack.ap())
            rowsT = pool.tile([4, 2 * D], F32, tag="rowsT")
            nc.scalar.dma_start(rowsT[:], rows.ap())
            Ws = []
            for i in range(4):
                Wi = pool.tile([128, 385], FP16, tag=f"W{i}")
                nc.sync.dma_start(Wi[0:64, :],
                                  AP(b2h, 512 * i, [[1, 64], [1, 385]]))
                nc.scalar.dma_start(Wi[64:128, :],
                                    AP(b2h, 512 * i + 64, [[1, 64], [1, 385]]))
                Ws.append(Wi)

            # --- PE warmup: the HAM modulator grants full speed after
            # ~3.4us of CONTINUOUS activity, and re-throttles after ~3.4us
            # idle.  Eight 512-col dummies open the gate; one more keyed on
            # the rows DMA (~8.5us) bridges the gap to the conv matmuls.
            Pd = psum.tile([4, 2 * D], F32, tag="Pd")
            for j in range(8):
                nc.tensor.matmul(
                    Pd[:], dmy[:, 0:4], dmy[:],
                    start=(j == 0), stop=(j == 7),
                )
            Pd2 = psum.tile([4, 2 * D], F32, tag="Pd2")
            nc.tensor.matmul(Pd2[:], rowsT[0:4, 0:4], rowsT[:],
                             start=True, stop=True)

            # --- ACT: rows exp (stats), a-side exp straight into the
            # block-diagonal lhsT (block k=2i+c holds ea_i chunk c in its
            # column i => col 9i+4c), then ONE strided exp over all four
            # conv windows.
            erows = pool.tile([4, 2 * D], F32, tag="erows")
            nc.scalar.activation(erows[:], rowsT[:], EXP, bias=bias4[:],
                                 scale=10.0)
            a0 = apackT[:, 0:1]
            b0 = bd[:, 0:1]
            nc.scalar.activation(
                AP(b0.tensor, b0.offset, [list(b0.ap[0]), [4, 2], [9, 4]]),
                AP(a0.tensor, a0.offset, [list(a0.ap[0]), [4, 2], [1, 4]]),
                EXP, bias=bias128[:], scale=10.0,
            )
            Wcat = pool.tile([128, 4 * 385], BF16, tag="Wcat")
            for i in range(4):
                nc.scalar.activation(
                    Wcat[:, 385 * i:385 * i + 385], Ws[i][:],
                    EXP, bias=bias128[:], scale=10.0,
                )

            # --- stats, all from the exp'd rows.  One inclusive cumsum over
            # the full [a_rev | b] row gives sum(ea) = Pref[255], sum(eb) =
            # Pref[511] - Pref[255], and the p1 prefix dot shifts by
            # sum(ea)^2: p1 = Z - dot(ea_rev, Pref[256:512]) + sum(ea)^2.
            # stg1 columns 0 (Z), 32 (m1 = p1 + u255/2), 64 (u255) feed one
            # transpose into row space for the carry chain.
            Pref = pool.tile([4, 2 * D], F32, tag="Pref")
            nc.vector.tensor_tensor_scan(
                Pref[:], erows[:], erows[:], 0.0,
                op0=ADD, op1=BYPASS,
            )
            p1x = pool.tile([4, 1], F32, tag="p1x")
            wscr = pool.tile([4, D], F32, tag="wscr")
            nc.vector.scalar_tensor_tensor(
                wscr[:], erows[:, 0:D], 1.0, Pref[:, D:2 * D],
                op0=MULT, op1=MULT, accum_out=p1x[:],
            )
            wscr2 = pool.tile([4, D], F32, tag="wscr2")
            nc.vector.scalar_tensor_tensor(
                wscr2[:], erows[:, 0:D], 1.0, erows[:, D:2 * D],
                op0=MULT, op1=MULT, accum_out=stg1[0:4, 64:65],
            )
            # c1 = sum(ea)*total, Z = c1 - sum(ea)^2, p1 = c1 - dotp
            # (the sum(ea)^2 prefix-shift cancels inside p1)
            c1 = pool.tile([4, 1], F32, tag="c1")
            nc.gpsimd.tensor_mul(c1[:], Pref[:, 255:256], Pref[:, 511:512])
            sqa = pool.tile([4, 1], F32, tag="sqa")
            nc.gpsimd.tensor_mul(sqa[:], Pref[:, 255:256], Pref[:, 255:256])
            nc.gpsimd.tensor_sub(stg1[0:4, 0:1], c1[:], sqa[:])
            p1c = pool.tile([4, 1], F32, tag="p1c")
            nc.gpsimd.tensor_sub(p1c[:], c1[:], p1x[:])
            uh = pool.tile([4, 1], F32, tag="uh")
            nc.gpsimd.tensor_scalar_mul(uh[:], stg1[0:4, 64:65], 0.5)
            nc.gpsimd.tensor_add(stg1[0:4, 32:33], p1c[:], uh[:])

            stg1T = pool.tile([32, 96], F32, tag="stg1T")
            nc.vector.transpose(stg1T[:], stg1[:])
            Zrow = stg1T[0:1, 0:4]
            m1row = stg1T[0:1, 32:36]
            u255row = stg1T[0:1, 64:68]

            # A'' = 10*m1/Z - 5 ; B'' = 5*u255/Z (tanh-halved recurrence)
            zr4 = pool.tile([1, 4], F32, tag="zr4")
            nc.vector.reciprocal(zr4[:], Zrow)
            ta = pool.tile([1, 4], F32, tag="ta")
            nc.vector.tensor_mul(ta[:], m1row, zr4[:])
            A2 = pool.tile([1, 4], F32, tag="A2")
            nc.vector.tensor_scalar(A2[:], ta[:], 10.0, -5.0,
                                    op0=MULT, op1=ADD)
            tb = pool.tile([1, 4], F32, tag="tb")
            nc.gpsimd.tensor_mul(tb[:], u255row, zr4[:])
            B2 = pool.tile([1, 4], F32, tag="B2")
            nc.gpsimd.tensor_scalar_mul(B2[:], tb[:], 5.0)
            # hrow = 1/(2Z) row, for the alpha/beta blend scalars
            hrow = pool.tile([1, 4], F32, tag="hrow")
            nc.gpsimd.tensor_scalar_mul(hrow[:], zr4[:], 0.5)

            # --- carry chain: t_i = tanh(B''_{i-1} t_{i-1} + A''_{i-1}),
            # one ACTIVATE per step (t_0 is the compile-time constant)
            for i in range(1, 4):
                nc.scalar.activation(stg2[0:1, i:i + 1],
                                     stg2[0:1, i - 1:i], TANH,
                                     bias=A2[0:1, i - 1:i],
                                     scale=B2[0:1, i - 1:i])

            # --- conv matmuls: P_u[i, 1+j] = u_i[j], P_u[i, 0] = u_i[255]
            P_u = psum.tile([4, 257], F32, tag="P_u")
            for i in range(4):
                for c in range(2):
                    k = 2 * i + c
                    base = 385 * i + 128 * c
                    nc.tensor.matmul(
                        P_u[:], bd[:, 4 * k:4 * k + 4],
                        Wcat[:, base:base + 257],
                        start=(k == 0), stop=(k == 7),
                    )

            # --- keepalive: a dummy DMA keyed on the chain keeps the sync
            # HWDGE queue awake so the output DMA skips the ~1us queue wake
            scr = pool.tile([1, 4], F32, tag="scr")
            nc.sync.dma_start(scr[:], A2[:])

            # alpha = (1-t)/(2Z), beta = (1+t)/(2Z): row-space TTs into the
            # two staging rows, then ONE transpose to per-partition columns.
            stgG = pool.tile([32, 64], F32, tag="stgG")
            nc.vector.memset(stgG[:], 0.0)
            trow = stg2[0:1, 0:4]
            th = pool.tile([1, 4], F32, tag="th")
            nc.vector.tensor_mul(th[:], trow, hrow[:])
            nc.gpsimd.tensor_sub(stgG[0:1, 0:4], hrow[:], th[:])
            nc.vector.tensor_add(stgG[0:1, 32:36], hrow[:], th[:])
            stgGT = pool.tile([32, 64], F32, tag="stgGT")
            nc.vector.transpose(stgGT[:], stgG[:])
            alT = stgGT[0:4, 0:1]
            beT = stgGT[0:4, 32:33]

            # --- combine: o1 = alpha*u ; out = beta*roll + o1
            o1 = pool.tile([4, D], F32, tag="o1")
            nc.vector.scalar_tensor_tensor(
                o1[:], P_u[:, 1:257], alT, zrow[:],
                op0=MULT, op1=ADD,
            )
            oout = pool.tile([4, D], FP16, tag="oout")
            nc.vector.scalar_tensor_tensor(
                oout[:], P_u[:, 0:256], beT, o1[:],
                op0=MULT, op1=ADD,
            )
            nc.sync.dma_start(out.ap(), oout[:])

    nc.compile()
    return nc


def prep_inputs(a_emb, b_emb):
    a = np.ascontiguousarray(a_emb, dtype=np.float32)
    b = np.ascontiguousarray(b_emb, dtype=np.float32)
    arev = np.ascontiguousarray(a[:, ::-1])                  # [4, 256]
    arevT = arev.T                                           # [256, 4]
    apack = np.empty((128, 8), np.float32)
    for c in range(2):
        apack[:, 4 * c:4 * c + 4] = arevT[128 * c:128 * (c + 1)]
    b16 = b.astype(np.float16)
    b2h = np.concatenate([b16, b16], axis=1)                 # [4, 512] fp16
    # stats rows use the SAME encodings the conv sees: fp32 a_rev (matches
    # apack) and fp32(fp16(b)) (matches b2h), so u/Z stay consistent.
    rows = np.concatenate([arev, b16.astype(np.float32)], axis=1)
    return {"apack": np.ascontiguousarray(apack),
            "rows": np.ascontiguousarray(rows),
            "b2h": np.ascontiguousarray(b2h)}


_NC_CACHE = {}


def run(a_emb, b_emb, trace=False):
    if "nc" not in _NC_CACHE:
        _NC_CACHE["nc"] = build_nc()
    nc = _NC_CACHE["nc"]
    in_map = prep_inputs(a_emb, b_emb)
    res = run_bass_kernel_spmd(
        nc, [in_map] * N_CORES, core_ids=list(range(N_CORES)), trace=trace
    )
    return np.asarray(res.results[0]["out"], dtype=np.float32), res


NUM_ENTRIES = 256 * 256 * 2


def _tables_match(W1, W2_sum, W2_carry):
    """Exact structural check of the deterministic one-hot tables."""
    try:
        W1 = np.asarray(W1)
        W2s = np.asarray(W2_sum)
        W2c = np.asarray(W2_carry)
        if (W1.shape != (514, NUM_ENTRIES) or W2s.shape != (NUM_ENTRIES, 256)
                or W2c.shape != (NUM_ENTRIES, 2)):
            return False
        idx = np.arange(NUM_ENTRIES)
        a = idx // 512
        b = (idx % 512) // 2
        c = idx % 2
        total = a + b + c
        # probed positions must be exactly 1 and |sum| must equal the count,
        # which (with the probes) pins every other entry to exactly 0
        if not (np.abs(W1).sum() == 3.0 * NUM_ENTRIES
                and (W1[a, idx] == 1.0).all()
                and (W1[256 + b, idx] == 1.0).all()
                and (W1[512 + c, idx] == 1.0).all()):
            return False
        if not (np.abs(W2s).sum() == float(NUM_ENTRIES)
                and (W2s[idx, total & 255] == 1.0).all()):
            return False
        if not (np.abs(W2c).sum() == float(NUM_ENTRIES)
                and (W2c[idx, (total >= 256).astype(np.int64)] == 1.0).all()):
            return False
        return True
    except Exception:
        return False


def _fallback_jax(a_emb, b_emb, W1, W2_sum, W2_carry):
    """Direct evaluation of the reference on the neuron devices via jax.
    Only reached if the tables are not the deterministic one-hot structure."""
    import jax
    import jax.numpy as jnp

    def step(carry, ab):
        a_i, b_i = ab
        x = jnp.concatenate([a_i, b_i, carry])
        scores = x @ jnp.asarray(W1)
        weights = jax.nn.softmax((scores - 2.5) * 10.0)
        return weights @ jnp.asarray(W2_carry), weights @ jnp.asarray(W2_sum)

    carry0 = jnp.zeros(2, dtype=jnp.float32).at[0].set(1.0)
    _, results = jax.lax.scan(
        step, carry0, (jnp.asarray(a_emb), jnp.asarray(b_emb))
    )
    return np.asarray(results, dtype=np.float32)


def kernel(a_emb, b_emb, W1, W2_sum, W2_carry):
    if not _tables_match(W1, W2_sum, W2_carry):
        return _fallback_jax(a_emb, b_emb, W1, W2_sum, W2_carry)
    o, _ = run(a_emb, b_emb, trace=False)
    return o


# revision 14
# speedup vs baseline: 1.0664x; 1.0664x over previous
"""Trainium2 Bass kernel for nn_AdditionFFN (4-step byte-addition FFN).

Reference semantics: 4 sequential steps; step i forms x = [a_i, b_i, carry]
(len 514), takes softmax(10*(x @ W1 - 2.5)) over 131072 one-hot table
entries, then result_i = weights @ W2_sum and carry' = weights @ W2_carry.

The tables are the deterministic one-hot structures from the reference's
_build_tables() (entry idx = a*512 + b*2 + c scores
a_emb[i,a] + b_emb[i,b] + carry[c]); kernel() verifies this structure
exactly and falls back to a direct on-device jax evaluation if it ever
fails to hold.  Under that structure the softmax factorizes:

    exp-scores = (ea (x) eb) (x) [e^{c0}, e^{c1}],  ea/eb = exp(10*emb - 12.5)

so weights @ W2_sum collapses to a 256-point circular convolution
u = ea (*) eb plus a roll by one for the odd-parity (carry-in) half, and
the carry chain reduces to a scalar recurrence over t_i = tanh(s_i/2)
(sigma_i = (1+t_i)/2 is the carry-in-1 mass):

    Z_i   = sum(ea_i) * sum(eb_i)            (= sum(u_i))
    p1_i  = sum_{a+b>=256} ea[a] eb[b]       (carry-out mass, c=0 part)
    A''_i = (10*p1_i + 5*u_i[255])/Z_i - 5 ;  B''_i = 5*u_i[255]/Z_i
    t_{i+1} = tanh(B''_i * t_i + A''_i),      t_0 = tanh(-5)
    out_i = (1-sg_i)/Z_i * u_i + sg_i/Z_i * roll(u_i, 1),  sg = (1+t)/2

Device implementation (SPMD on 8 NeuronCores, no collectives - every core
computes the full answer redundantly and core 0's output is returned):

  - circular conv as TensorE correlation: lhsT row q = ea[255-q]
    (block-diagonal bf16 [128, 4] chunks, written DIRECTLY by the a-side
    exp through a strided output AP); one overlapping-run DMA of the
    host-cast fp16 [4, 512] copy gives Wall[p, m] = flat[p + m], so ONE
    strided ACTIVATE exps all four step windows at once.  The psum is
    [4, 257] wide: col j = u[(j-1) mod 256] - roll(u,1) is P[:, 0:256]
    and u is P[:, 1:257] with no wrap fixup.
  - ALL softmax statistics come from host-packed rows R = [a_rev | b]
    ([4, 512], exp'd once on ACT): u255 = dot(ea_rev, eb_row) (Pool STT
    accumulate), PrefB = inclusive cumsum of eb_row (DVE scan),
    p1 = Z - dot(ea_rev, PrefB), Z = sum(ea) * PrefB[255].  No second
    matmul group and no on-device one-hot tables at all.
  - carry chain: t' = tanh(B''*t + A'') folds each recurrence step into a
    single ACTIVATE (per-instruction scale/bias APs); tanh lives in the
    same ACT table set as exp, so there is exactly one table load.
  - final combine is two DVE STTs with per-partition scalars prescaled by
    1/Z on the idle Pool engine: alpha = (1-sg)/(2...)/Z, beta = sg/Z:
    o1 = alpha*u, out = beta*roll + o1 (PSUM is read once per STT, which
    the s2s2d2 datapath requires anyway).
  - six back-to-back [4,512] dummy matmuls open the PE HAM clock gate
    (~3.4us of continuous activity) so the 8 real conv matmuls run warm.
  - a dummy DMA keyed on the chain keeps the sync HWDGE queue awake so
    the output DMA skips the ~1us queue wake.
"""

import sys

sys.path.insert(0, "/opt/trn_rl_repo")

import numpy as np

import concourse.bacc as bacc
import concourse.mybir as mybir
import concourse.tile as tile
from concourse.ap import AP
from concourse.bass_utils import run_bass_kernel_spmd

N_CORES = 8
D = 256
F32 = mybir.dt.float32
BF16 = mybir.dt.bfloat16
FP16 = mybir.dt.float16
EXP = mybir.ActivationFunctionType.Exp
TANH = mybir.ActivationFunctionType.Tanh
MULT = mybir.AluOpType.mult
ADD = mybir.AluOpType.add
BYPASS = mybir.AluOpType.bypass
T0 = float(np.tanh(-5.0))


def build_nc():
    nc = bacc.Bacc(None, target_bir_lowering=False, debug=False,
                   enable_partition_id=False)

    apack = nc.declare_dram_parameter("apack", [128, 8], F32, isOutput=False)
    rows = nc.declare_dram_parameter("rows", [4, 2 * D], F32, isOutput=False)
    b2h = nc.declare_dram_parameter("b2h", [4, 2 * D], FP16, isOutput=False)
    out = nc.declare_dram_parameter("out", [4, D], FP16, isOutput=True)

    with tile.TileContext(nc) as tc:
        with (
            tc.tile_pool(name="pool", bufs=1) as pool,
            tc.tile_pool(name="psum", bufs=1, space="PSUM") as psum,
        ):
            # --- constants
            bd = pool.tile([128, 32], BF16, tag="bd")
            nc.gpsimd.memset(bd[:], 0.0)
            bias128 = pool.tile([128, 1], F32, tag="bias128")
            nc.vector.memset(bias128[:], -12.5)
            bias4 = pool.tile([4, 1], F32, tag="bias4")
            nc.vector.memset(bias4[:], -12.5)
            stg1 = pool.tile([32, 96], F32, tag="stg1")
            nc.vector.memset(stg1[:], 0.0)
            stg2 = pool.tile([32, 32], F32, tag="stg2")
            nc.vector.memset(stg2[:], 0.0)
            nc.vector.memset(stg2[0:1, 0:1], T0)
            zrow = pool.tile([4, D], F32, tag="zrow")
            nc.vector.memset(zrow[:], 0.0)

            # --- input DMAs: two parallel HWDGE queues.  Queue first-byte
            # is ~1.7us after dispatch and each dispatch occupies the
            # issuing engine's stream for 0.6-1.2us, so: exactly two bulk
            # window transfers (steps 0-1 on sync, steps 2-3 on scalar,
            # Wall[p, m] = flat[off + p + m]) behind one tiny tensor each.
            # The scalar-queue dispatches retire before the ACT table load
            # finishes, so ACT compute is never blocked by them.
            apackT = pool.tile([128, 8], F32, tag="apackT")
            nc.sync.dma_start(apackT[:], apack.ap())
            rowsT = pool.tile([4, 2 * D], F32, tag="rowsT")
            nc.scalar.dma_start(rowsT[:], rows.ap())
            WallA = pool.tile([128, 897], FP16, tag="WallA")
            WallB = pool.tile([128, 897], FP16, tag="WallB")
            nc.sync.dma_start(WallA[:], AP(b2h, 0, [[1, 128], [1, 897]]))
            nc.scalar.dma_start(WallB[:], AP(b2h, 1024, [[1, 128], [1, 897]]))

            # --- ACT: rows exp (stats), a-side exp straight into the
            # block-diagonal lhsT (block k=2i+c holds ea_i chunk c in its
            # column i => col 9i+4c), then ONE strided exp over all four
            # conv windows.
            erows = pool.tile([4, 2 * D], F32, tag="erows")
            nc.scalar.activation(erows[:], rowsT[:], EXP, bias=bias4[:],
                                 scale=10.0)
            a0 = apackT[:, 0:1]
            b0 = bd[:, 0:1]
            nc.scalar.activation(
                AP(b0.tensor, b0.offset, [list(b0.ap[0]), [4, 2], [9, 4]]),
                AP(a0.tensor, a0.offset, [list(a0.ap[0]), [4, 2], [1, 4]]),
                EXP, bias=bias128[:], scale=10.0,
            )
            Wcat = pool.tile([128, 4 * 385], BF16, tag="Wcat")
            for h, wall in enumerate((WallA, WallB)):
                wb = wall[:, 0:1]
                nc.scalar.activation(
                    Wcat[:, 770 * h:770 * h + 770].rearrange(
                        "p (i f) -> p i f", i=2),
                    AP(wb.tensor, wb.offset,
                       [list(wb.ap[0]), [512, 2], [1, 385]]),
                    EXP, bias=bias128[:], scale=10.0,
                )

            # --- stats, all from the exp'd rows.  One inclusive cumsum over
            # the full [a_rev | b] row gives sum(ea) = Pref[255], sum(eb) =
            # Pref[511] - Pref[255], and the p1 prefix dot shifts by
            # sum(ea)^2: p1 = Z - dot(ea_rev, Pref[256:512]) + sum(ea)^2.
            # stg1 columns 0 (Z), 32 (m1 = p1 + u255/2), 64 (u255) feed one
            # transpose into row space for the carry chain.
            Pref = pool.tile([4, 2 * D], F32, tag="Pref")
            nc.vector.tensor_tensor_scan(
                Pref[:], erows[:], erows[:], 0.0,
                op0=ADD, op1=BYPASS,
            )
            p1x = pool.tile([4, 1], F32, tag="p1x")
            wscr = pool.tile([4, D], F32, tag="wscr")
            nc.vector.scalar_tensor_tensor(
                wscr[:], erows[:, 0:D], 1.0, Pref[:, D:2 * D],
                op0=MULT, op1=MULT, accum_out=p1x[:],
            )
            wscr2 = pool.tile([4, D], F32, tag="wscr2")
            nc.vector.scalar_tensor_tensor(
                wscr2[:], erows[:, 0:D], 1.0, erows[:, D:2 * D],
                op0=MULT, op1=MULT, accum_out=stg1[0:4, 64:65],
            )
            # c1 = sum(ea)*total, Z = c1 - sum(ea)^2, p1 = c1 - dotp
            # (the sum(ea)^2 prefix-shift cancels inside p1)
            c1 = pool.tile([4, 1], F32, tag="c1")
            nc.gpsimd.tensor_mul(c1[:], Pref[:, 255:256], Pref[:, 511:512])
            sqa = pool.tile([4, 1], F32, tag="sqa")
            nc.gpsimd.tensor_mul(sqa[:], Pref[:, 255:256], Pref[:, 255:256])
            nc.gpsimd.tensor_sub(stg1[0:4, 0:1], c1[:], sqa[:])
            p1c = pool.tile([4, 1], F32, tag="p1c")
            nc.gpsimd.tensor_sub(p1c[:], c1[:], p1x[:])
            uh = pool.tile([4, 1], F32, tag="uh")
            nc.gpsimd.tensor_scalar_mul(uh[:], stg1[0:4, 64:65], 0.5)
            nc.gpsimd.tensor_add(stg1[0:4, 32:33], p1c[:], uh[:])

            stg1T = pool.tile([32, 96], F32, tag="stg1T")
            nc.vector.transpose(stg1T[:], stg1[:])
            Zrow = stg1T[0:1, 0:4]
            m1row = stg1T[0:1, 32:36]
            u255row = stg1T[0:1, 64:68]

            # A'' = 10*m1/Z - 5 ; B'' = 5*u255/Z (tanh-halved recurrence)
            zr4 = pool.tile([1, 4], F32, tag="zr4")
            nc.vector.reciprocal(zr4[:], Zrow)
            ta = pool.tile([1, 4], F32, tag="ta")
            nc.vector.tensor_mul(ta[:], m1row, zr4[:])
            A2 = pool.tile([1, 4], F32, tag="A2")
            nc.vector.tensor_scalar(A2[:], ta[:], 10.0, -5.0,
                                    op0=MULT, op1=ADD)
            tb = pool.tile([1, 4], F32, tag="tb")
            nc.gpsimd.tensor_mul(tb[:], u255row, zr4[:])
            B2 = pool.tile([1, 4], F32, tag="B2")
            nc.gpsimd.tensor_scalar_mul(B2[:], tb[:], 5.0)
            # hrow = 1/(2Z) row, for the alpha/beta blend scalars
            hrow = pool.tile([1, 4], F32, tag="hrow")
            nc.gpsimd.tensor_scalar_mul(hrow[:], zr4[:], 0.5)

            # --- carry chain: t_i = tanh(B''_{i-1} t_{i-1} + A''_{i-1}),
            # one ACTIVATE per step (t_0 is the compile-time constant)
            for i in range(1, 4):
                nc.scalar.activation(stg2[0:1, i:i + 1],
                                     stg2[0:1, i - 1:i], TANH,
                                     bias=A2[0:1, i - 1:i],
                                     scale=B2[0:1, i - 1:i])

            # --- conv matmuls: P_u[i, 1+j] = u_i[j], P_u[i, 0] = u_i[255]
            P_u = psum.tile([4, 257], F32, tag="P_u")
            for i in range(4):
                for c in range(2):
                    k = 2 * i + c
                    base = 385 * i + 128 * c
                    nc.tensor.matmul(
                        P_u[:], bd[:, 4 * k:4 * k + 4],
                        Wcat[:, base:base + 257],
                        start=(k == 0), stop=(k == 7),
                    )

            # --- keepalive: a dummy DMA keyed on the chain keeps the sync
            # HWDGE queue awake so the output DMA skips the ~1us queue wake
            scr = pool.tile([1, 4], F32, tag="scr")
            nc.sync.dma_start(scr[:], A2[:])

            # alpha = (1-t)/(2Z), beta = (1+t)/(2Z): row-space TTs into the
            # two staging rows, then ONE transpose to per-partition columns.
            stgG = pool.tile([32, 64], F32, tag="stgG")
            nc.vector.memset(stgG[:], 0.0)
            trow = stg2[0:1, 0:4]
            th = pool.tile([1, 4], F32, tag="th")
            nc.vector.tensor_mul(th[:], trow, hrow[:])
            nc.gpsimd.tensor_sub(stgG[0:1, 0:4], hrow[:], th[:])
            nc.vector.tensor_add(stgG[0:1, 32:36], hrow[:], th[:])
            stgGT = pool.tile([32, 64], F32, tag="stgGT")
            nc.vector.transpose(stgGT[:], stgG[:])
            alT = stgGT[0:4, 0:1]
            beT = stgGT[0:4, 32:33]

            # --- combine: o1 = alpha*u ; out = beta*roll + o1
            o1 = pool.tile([4, D], F32, tag="o1")
            nc.vector.scalar_tensor_tensor(
                o1[:], P_u[:, 1:257], alT, zrow[:],
                op0=MULT, op1=ADD,
            )
            oout = pool.tile([4, D], FP16, tag="oout")
            nc.vector.scalar_tensor_tensor(
                oout[:], P_u[:, 0:256], beT, o1[:],
                op0=MULT, op1=ADD,
            )
            nc.sync.dma_start(out.ap(), oout[:])

    nc.compile()
    return nc


def prep_inputs(a_emb, b_emb):
    a = np.ascontiguousarray(a_emb, dtype=np.float32)
    b = np.ascontiguousarray(b_emb, dtype=np.float32)
    arev = np.ascontiguousarray(a[:, ::-1])                  # [4, 256]
    arevT = arev.T                                           # [256, 4]
    apack = np.empty((128, 8), np.float32)
    for c in range(2):
        apack[:, 4 * c:4 * c + 4] = arevT[128 * c:128 * (c + 1)]
    b16 = b.astype(np.float16)
    b2h = np.concatenate([b16, b16], axis=1)                 # [4, 512] fp16
    # stats rows use the SAME encodings the conv sees: fp32 a_rev (matches
    # apack) and fp32(fp16(b)) (matches b2h), so u/Z stay consistent.
    rows = np.concatenate([arev, b16.astype(np.float32)], axis=1)
    return {"apack": np.ascontiguousarray(apack),
            "rows": np.ascontiguousarray(rows),
            "b2h": np.ascontiguousarray(b2h)}


_NC_CACHE = {}


def run(a_emb, b_emb, trace=False):
    if "nc" not in _NC_CACHE:
        _NC_CACHE["nc"] = build_nc()
    nc = _NC_CACHE["nc"]
    in_map = prep_inputs(a_emb, b_emb)
    res = run_bass_kernel_spmd(
        nc, [in_map] * N_CORES, core_ids=list(range(N_CORES)), trace=trace
    )
    return np.asarray(res.results[0]["out"], dtype=np.float32), res


NUM_ENTRIES = 256 * 256 * 2


def _tables_match(W1, W2_sum, W2_carry):
    """Exact structural check of the deterministic one-hot tables."""
    try:
        W1 = np.asarray(W1)
        W2s = np.asarray(W2_sum)
        W2c = np.asarray(W2_carry)
        if (W1.shape != (514, NUM_ENTRIES) or W2s.shape != (NUM_ENTRIES, 256)
                or W2c.shape != (NUM_ENTRIES, 2)):
            return False
        idx = np.arange(NUM_ENTRIES)
        a = idx // 512
        b = (idx % 512) // 2
        c = idx % 2
        total = a + b + c
        # probed positions must be exactly 1 and |sum| must equal the count,
        # which (with the probes) pins every other entry to exactly 0
        if not (np.abs(W1).sum() == 3.0 * NUM_ENTRIES
                and (W1[a, idx] == 1.0).all()
                and (W1[256 + b, idx] == 1.0).all()
                and (W1[512 + c, idx] == 1.0).all()):
            return False
        if not (np.abs(W2s).sum() == float(NUM_ENTRIES)
                and (W2s[idx, total & 255] == 1.0).all()):
            return False
        if not (np.abs(W2c).sum() == float(NUM_ENTRIES)
                and (W2c[idx, (total >= 256).astype(np.int64)] == 1.0).all()):
            return False
        return True
    except Exception:
        return False


def _fallback_jax(a_emb, b_emb, W1, W2_sum, W2_carry):
    """Direct evaluation of the reference on the neuron devices via jax.
    Only reached if the tables are not the deterministic one-hot structure."""
    import jax
    import jax.numpy as jnp

    def step(carry, ab):
        a_i, b_i = ab
        x = jnp.concatenate([a_i, b_i, carry])
        scores = x @ jnp.asarray(W1)
        weights = jax.nn.softmax((scores - 2.5) * 10.0)
        return weights @ jnp.asarray(W2_carry), weights @ jnp.asarray(W2_sum)

    carry0 = jnp.zeros(2, dtype=jnp.float32).at[0].set(1.0)
    _, results = jax.lax.scan(
        step, carry0, (jnp.asarray(a_emb), jnp.asarray(b_emb))
    )
    return np.asarray(results, dtype=np.float32)


def kernel(a_emb, b_emb, W1, W2_sum, W2_carry):
    if not _tables_match(W1, W2_sum, W2_carry):
        return _fallback_jax(a_emb, b_emb, W1, W2_sum, W2_carry)
    o, _ = run(a_emb, b_emb, trace=False)
    return o
